# revision 44
# baseline (speedup 1.0000x reference)
"""Causal multi-head attention block (qkv proj + attention + out proj) on 8
Trainium2 NeuronCores.

Sharding: core c = 2*b + hg handles batch b (of 4) and head-group hg (8 of 16
heads).  Each core computes qkv for its heads, causal attention, and a partial
out-projection (its 512 rows of w_out); the host sums the two head-group
partials per batch.

Design:
  - host pre-transposes x to xT [DIM, T] and pre-casts to bf16: no PE
    transposes on device, half the input DMA bytes.
  - qkv projection, scores and out_proj run in bf16 (1 cycle/row at every
    p-state and free size).  P@V runs in fp8e4m3 for q-blocks 1-3 — DoubleRow
    perf mode on off-diagonal k-tile pairs (two 128-row contraction slabs per
    matmul at 0.5 cycles/row) and plain fp8 on diagonal tiles — and in bf16
    for q-block 0, whose short attention rows would amplify fp8 quantization
    past the error budget (verified by simulation: hybrid 4.3e-3 vs 2.8e-2
    for fp8-everywhere, gate 2e-2).
  - V is stored 16x-scaled in the fp8 tiles (subnormal dodge) with a 16.0
    ones column, which cancels exactly in the softmax ratio; exp applies
    bias -3.25 so the max P (max causal score is 66.5 -> logit 8.3) stays
    under fp8e4m3's 240.
  - softmax normalization: DVE copies evacuate the PSUM accumulator fast,
    the reciprocal runs 8 elems/lane via a DRAM reshape on the idle gpsimd
    SWDGE queue, and the divide doubles as the f32->bf16 cast for out_proj.
  - emission is phase-interleaved: qkv quarter q+1 / out_proj q-1 (dense PE
    work) pump into attention block q (ACT-heavy) to keep the PE dense and
    p-state ramped; input DMA is split across both HWDGE queues with
    first-needed tiles first.
"""

import sys

if "/opt/trn_rl_repo" not in sys.path:
    sys.path.insert(0, "/opt/trn_rl_repo")

import numpy as np
import ml_dtypes

import concourse.bass as bass
import concourse.mybir as mybir
import concourse.tile as tile
from concourse import bacc
from concourse.bass_utils import run_bass_kernel_spmd

# All our activations (Exp, Ln, Copy) live in the natural_log_exp_and_others
# table set. By default the table-load pass maps Exp to exp_and_others (first
# set containing it) and Ln to natural_log_*, inserting two 1.3us table
# reloads into the latency-critical tail. Restrict the table map so every
# activation resolves to the shared set: one load at startup, none later.
_orig_get_act_tables = bacc.get_activation_tables


def _patched_get_act_tables(arch):
    tabs = _orig_get_act_tables(arch)
    keep = "natural_log_exp_and_others"
    if keep in tabs:
        ours = {mybir.ActivationFunctionType.Exp,
                mybir.ActivationFunctionType.Ln,
                mybir.ActivationFunctionType.Copy}
        tabs = {name: (fns if name == keep else fns - ours)
                for name, fns in tabs.items()}
    return tabs


bacc.get_activation_tables = _patched_get_act_tables

DIM = 1024
N_HEAD = 16
HD = 64
B, T = 4, 2048
HG = 8          # heads per core
CQ = HG * HD    # 512 feature columns per group
NCORES = 8
NT = T // 128   # 16 t-subtiles
NQ = T // 512   # 4 quarters / q-blocks

f32 = mybir.dt.float32
bf16 = mybir.dt.bfloat16
f8 = mybir.dt.float8e4
Exp = mybir.ActivationFunctionType.Exp
DR = mybir.MatmulPerfMode.DoubleRow
EXP_BIAS = -3.25


def build_nc():
    nc = bacc.Bacc(None, target_bir_lowering=False)
    # bf16 x columns t 0..511 only (quarter 0); rows t>=512 project in fp8
    xt_d = nc.declare_dram_parameter("xt", [DIM, 512], bf16, isOutput=False)
    xt8_d = nc.declare_dram_parameter("xt8", [DIM, T - 512], f8, isOutput=False)
    wqk_d = nc.declare_dram_parameter("wqk", [DIM, 2 * CQ], bf16, isOutput=False)
    wqk8_d = nc.declare_dram_parameter("wqk8", [DIM, 2 * CQ], f8,
                                       isOutput=False)
    wv_d = nc.declare_dram_parameter("wv", [DIM, CQ], bf16, isOutput=False)
    wv8_d = nc.declare_dram_parameter("wv8", [DIM, CQ], f8, isOutput=False)
    wo_d = nc.declare_dram_parameter("wo", [CQ, DIM], bf16, isOutput=False)
    mv_d = nc.declare_dram_parameter("maskv", [128, NT], f32, isOutput=False)
    out_d = nc.declare_dram_parameter("out", [T, DIM], bf16, isOutput=True)

    with tile.TileContext(nc) as tc:
        with tc.tile_pool(name="pp", bufs=1) as pp, \
             tc.tile_pool(name="p_p", bufs=6) as p_p, \
             tc.tile_pool(name="p_pb", bufs=6) as p_pb, \
             tc.tile_pool(name="at_p", bufs=2) as at_p, \
             tc.tile_pool(name="dn_p", bufs=3) as dn_p, \
             tc.tile_pool(name="bcs_p", bufs=2) as bcs_p, \
             tc.tile_pool(name="out_p", bufs=4) as out_p, \
             tc.tile_pool(name="dram_p", bufs=3, space="DRAM") as dram_p, \
             tc.tile_pool(name="ps_aux", bufs=2, space="PSUM") as ps_aux, \
             tc.tile_pool(name="ps_s", bufs=2, space="PSUM") as ps_s, \
             tc.tile_pool(name="ps_pv", bufs=1, space="PSUM") as ps_pv:

            # ---- persistent input tiles ----
            # merged big tiles + per-k views: each input lands in ONE or TWO
            # big multi-tile DMAs. The scalar HWDGE queue gets only 5 trigger
            # instructions, so the first ACTIVATE isn't stuck behind a
            # 40-trigger FIFO (measured: exp start 46us -> ~16us).
            TQ = T - 512
            xbig = pp.tile([128, 8 * 512], bf16, name="xbig", tag="xbig")
            xts = [xbig[:, kb * 512:(kb + 1) * 512] for kb in range(8)]
            wqkbig = pp.tile([128, 8 * 2 * CQ], bf16, name="wqkbig",
                             tag="wqkbig")
            wqk_sb = [wqkbig[:, k * 2 * CQ:(k + 1) * 2 * CQ] for k in range(8)]
            wvbig = pp.tile([128, 8 * CQ], bf16, name="wvbig", tag="wvbig")
            wv_sb = [wvbig[:, k * CQ:(k + 1) * CQ] for k in range(8)]
            wobig = pp.tile([128, 4 * DIM], bf16, name="wobig", tag="wobig")
            wo_sb = [wobig[:, m * DIM:(m + 1) * DIM] for m in range(4)]
            mv_sb = pp.tile([128, NT], f32, name="maskv_sb", tag="maskv_sb")
            # fp8 DoubleRow pair tiles (dim-block pairs on the slab axis):
            # 16x-scaled weights; x rows t>=512
            x8big = pp.tile([128, 8 * TQ], f8, name="x8big", tag="x8big")
            x8v = [x8big[:, kp * 2 * TQ:(kp + 1) * 2 * TQ]
                   .rearrange("p (s c) -> p s c", c=TQ) for kp in range(4)]
            wqk8big = pp.tile([128, 8 * 2 * CQ], f8, name="wqk8big",
                              tag="wqk8big")
            wqk8v = [wqk8big[:, kp * 4 * CQ:(kp + 1) * 4 * CQ]
                     .rearrange("p (s c) -> p s c", c=2 * CQ)
                     for kp in range(4)]
            wv8big = pp.tile([128, 8 * CQ], f8, name="wv8big", tag="wv8big")
            wv8v = [wv8big[:, kp * 2 * CQ:(kp + 1) * 2 * CQ]
                    .rearrange("p (s c) -> p s c", c=CQ) for kp in range(4)]

            # big-tile (k, p, c) views for the batched DMAs
            xbig3 = xbig.rearrange("p (k c) -> p k c", c=512)
            xd3 = xt_d.rearrange("(k p) c -> p k c", p=128)
            wqkbig3 = wqkbig.rearrange("p (k c) -> p k c", c=2 * CQ)
            wqkd3 = wqk_d.rearrange("(k p) c -> p k c", p=128)
            wvbig3 = wvbig.rearrange("p (k c) -> p k c", c=CQ)
            wvd3 = wv_d.rearrange("(k p) c -> p k c", p=128)
            x8big3 = x8big.rearrange("p (j c) -> p j c", c=TQ)
            x8d3 = xt8_d.rearrange("(j p) c -> p j c", p=128)
            wqk8big3 = wqk8big.rearrange("p (j c) -> p j c", c=2 * CQ)
            wqk8d3 = wqk8_d.rearrange("(j p) c -> p j c", p=128)
            wv8big3 = wv8big.rearrange("p (j c) -> p j c", c=CQ)
            wv8d3 = wv8_d.rearrange("(j p) c -> p j c", p=128)

            # scalar queue: 5 early triggers only (maskv + its half of the
            # startup burst); everything else rides sync in first-needed order
            nc.scalar.dma_start(out=mv_sb, in_=mv_d[:, :])
            nc.sync.dma_start(out=wqkbig3[:, 0:4, 0:128],
                              in_=wqkd3[:, 0:4, 0:128])
            nc.scalar.dma_start(out=wqkbig3[:, 4:8, 0:128],
                                in_=wqkd3[:, 4:8, 0:128])
            nc.sync.dma_start(out=xbig3[:, 0:4, :], in_=xd3[:, 0:4, :])
            nc.scalar.dma_start(out=xbig3[:, 4:8, :], in_=xd3[:, 4:8, :])
            # k-projection m=4 columns next: the first score matmul needs
            # kt[0], which is unit m4's output
            nc.sync.dma_start(out=wqkbig3[:, 0:4, 512:640],
                              in_=wqkd3[:, 0:4, 512:640])
            nc.scalar.dma_start(out=wqkbig3[:, 4:8, 512:640],
                                in_=wqkd3[:, 4:8, 512:640])
            # wv before the wqk remainders: the v units are the first
            # phase-1 fillers
            nc.sync.dma_start(out=wvbig3[:, 0:4, :], in_=wvd3[:, 0:4, :])
            nc.scalar.dma_start(out=wvbig3[:, 4:8, :], in_=wvd3[:, 4:8, :])
            # wqk remainders all on sync: scalar's FIFO must clear so the
            # act-table load + first ACTIVATEs aren't queued behind triggers
            nc.sync.dma_start(out=wqkbig3[:, :, 128:512],
                              in_=wqkd3[:, :, 128:512])
            nc.sync.dma_start(out=wqkbig3[:, :, 640:2 * CQ],
                              in_=wqkd3[:, :, 640:2 * CQ])
            nc.sync.dma_start(out=wqk8big3[:, :, 0:128],
                              in_=wqk8d3[:, :, 0:128])
            nc.sync.dma_start(out=x8big3[:, :, 0:512], in_=x8d3[:, :, 0:512])
            nc.sync.dma_start(out=wqk8big3[:, :, 128:2 * CQ],
                              in_=wqk8d3[:, :, 128:2 * CQ])
            nc.sync.dma_start(out=wv8big3[:, :, :], in_=wv8d3[:, :, :])
            nc.sync.dma_start(out=x8big3[:, :, 512:TQ], in_=x8d3[:, :, 512:TQ])
            nc.sync.dma_start(out=wobig.rearrange("p (k c) -> p k c", c=DIM),
                              in_=wo_d.rearrange("(k p) c -> p k c", p=128))
            # dummy activation: hoists the 1.3us ACT table load into the
            # startup window instead of serializing before the first real exp
            actwarm = pp.tile([1, 1], f32, name="actwarm", tag="actwarm")
            nc.scalar.activation(actwarm, mv_sb[0:1, 0:1],
                                 mybir.ActivationFunctionType.Copy)

            # ---- constants ----
            # warm tile: the warm-up matmuls only need to occupy the PE
            # (outputs are discarded; emit_S start=True clears the PSUM banks
            # before real use). A 1-element DMA write satisfies the tile
            # allocator without a full memset on the critical path.
            warm = pp.tile([128, 512], bf16, name="warm", tag="warm")
            nc.sync.dma_start(out=warm[0:1, 0:1], in_=xt_d[0:1, 0:1])
            # one 128x128 causal strip: keep where q_local >= k_local
            dstrip32 = pp.tile([128, 128], f32, name="dstrip32", tag="dstrip32")
            nc.gpsimd.memset(dstrip32, 1.0)
            nc.gpsimd.affine_select(
                out=dstrip32, in_=dstrip32, compare_op=mybir.AluOpType.is_ge,
                fill=0.0, base=0, pattern=[[1, 128]], channel_multiplier=-1)
            dstrip8 = pp.tile([128, 128], f8, name="dstrip8", tag="dstrip8")
            nc.vector.tensor_copy(dstrip8, dstrip32)
            dstripb = pp.tile([128, 128], bf16, name="dstripb", tag="dstripb")
            nc.vector.tensor_copy(dstripb, dstrip32)
            # fp8 V path: values 16x-scaled, ones column 16.0
            ones16 = pp.tile([128, HG], f32, name="ones16", tag="ones16")
            nc.vector.memset(ones16, 16.0)
            ones1 = pp.tile([128, HG], f32, name="ones1", tag="ones1")
            nc.vector.memset(ones1, 1.0)
            mv16_sb = pp.tile([128, NT], f32, name="mv16", tag="mv16")
            nc.vector.tensor_scalar_mul(mv16_sb, mv_sb, 16.0)
            # exp bias (fp8e4m3 range headroom); cancels in the softmax
            nbias = pp.tile([128, 1], f32, name="nbias", tag="nbias")
            nc.vector.memset(nbias, EXP_BIAS)
            # [1, 64] ones row: K=1 matmul broadcasts 1/Z across partitions
            ones_row = pp.tile([1, 64], f32, name="ones_row", tag="ones_row")
            nc.vector.memset(ones_row, 1.0)

            # ---- persistent compute tensors ----
            kt = [pp.tile([128, T], bf16, name=f"kt{m}", tag=f"kt{m}")
                  for m in range(4)]
            # fp8 V tiles for DoubleRow: vaug2[kp] holds k-tiles (2kp, 2kp+1)
            # laid out (head, slab, 64 V cols + ones col + 15 pad): the slab
            # stride must be a 16B multiple for dual-fp8 ldweights
            vaug2 = [pp.tile([128, 2 * HG * 80], f8, name=f"va{t}", tag=f"va{t}")
                     for t in range(NT // 2)]
            # bf16 V tiles for q-block 0 (k-tiles 0-3 only), (head, 65) layout
            vaugb = [pp.tile([128, HG * 65], bf16, name=f"vb{t}", tag=f"vb{t}")
                     for t in range(4)]
            qt = [[pp.tile([128, 512], bf16, name=f"qt{q}_{m}", tag=f"qt{q}_{m}")
                   for m in range(4)] for q in range(NQ)]

            ats_cur = {}   # qb -> [4 pair tiles [128, 512] bf16]

            # ---------- qkv quarter units (each ~1.7us of PE) ----------
            # split=True emits each qk unit as two 4-matmul halves (the PSUM
            # accumulation group spans the split): finer filler grain in the
            # ACT-bound phase 3 so attention score matmuls queue sooner
            def qkv_units(q, split=False):
                units = []
                dr = q >= 1   # fp8 DoubleRow projection for rows t >= 512

                def qk_mms(pq, m, lo, hi):
                    if dr:
                        # halves map to slab pairs: (lo,hi) in MM units of 1
                        for kp in range(lo // 2, hi // 2):
                            nc.tensor.matmul(
                                pq, wqk8v[kp][:, :, m * 128:(m + 1) * 128],
                                x8v[kp][:, :, (q - 1) * 512:q * 512],
                                start=(kp == 0), stop=(kp == 3), perf_mode=DR)
                        return
                    # kb order matches DMA arrival (sync/scalar interleaved)
                    # so quarter 0 isn't gated on the whole startup burst
                    order = (0, 4, 1, 5, 2, 6, 3, 7)
                    for i in range(lo, hi):
                        nc.tensor.matmul(
                            pq, wqk_sb[order[i]][:, m * 128:(m + 1) * 128],
                            xts[order[i]][:, 0:512],
                            start=(i == 0), stop=(i == 7))

                def qk_fin(pq, m):
                    if m < 4:
                        nc.vector.tensor_copy(qt[q][m], pq)
                    else:
                        nc.vector.tensor_copy(
                            kt[m - 4][:, q * 512:(q + 1) * 512], pq)

                for m in range(8):
                    if split:
                        st = {}

                        def ha(m=m, st=st):
                            st["pq"] = ps_aux.tile(
                                [128, 512], f32, name="mm", tag="aux")
                            qk_mms(st["pq"], m, 0, 4)

                        def hb(m=m, st=st):
                            qk_mms(st["pq"], m, 4, 8)
                            qk_fin(st["pq"], m)
                        units.append(ha)
                        units.append(hb)
                    else:
                        def whole(m=m):
                            pq = ps_aux.tile(
                                [128, 512], f32, name="mm", tag="aux")
                            qk_mms(pq, m, 0, 8)
                            qk_fin(pq, m)
                        units.append(whole)

                def v_mms(pv, kti, lo, hi):
                    if dr:
                        for kp in range(lo // 2, hi // 2):
                            nc.tensor.matmul(
                                pv,
                                x8v[kp][:, :, (kti - 4) * 128:(kti - 3) * 128],
                                wv8v[kp], start=(kp == 0), stop=(kp == 3),
                                perf_mode=DR)
                        return
                    for kb in range(lo, hi):
                        nc.tensor.matmul(
                            pv, xts[kb][:, kti * 128:(kti + 1) * 128], wv_sb[kb],
                            start=(kb == 0), stop=(kb == 7))

                def v_fin(pv, kti):
                    pv3 = pv.rearrange("p (h w) -> p h w", w=64)
                    vt4 = vaug2[kti // 2].rearrange(
                        "p (h s u) -> p h s u", s=2, u=80)
                    s = kti % 2
                    # DR quarters produce 16x-scaled v already: mask only
                    vscale = mv_sb if dr else mv16_sb
                    nc.vector.tensor_scalar_mul(
                        vt4[:, :, s, 0:64], pv3, vscale[:, kti:kti + 1])
                    nc.vector.tensor_scalar_mul(
                        vt4[:, :, s, 64:65],
                        ones16.rearrange("p (h w) -> p h w", w=1),
                        mv_sb[:, kti:kti + 1])
                    if q == 0:
                        vb3 = vaugb[kti].rearrange("p (h u) -> p h u", u=65)
                        nc.vector.tensor_scalar_mul(
                            vb3[:, :, 0:64], pv3, mv_sb[:, kti:kti + 1])
                        nc.vector.tensor_scalar_mul(
                            vb3[:, :, 64:65],
                            ones1.rearrange("p (h w) -> p h w", w=1),
                            mv_sb[:, kti:kti + 1])

                for ti in range(4):
                    kti = q * 4 + ti
                    if split:
                        vst = {}

                        def va(kti=kti, vst=vst):
                            vst["pv"] = ps_aux.tile(
                                [128, 512], f32, name="mm", tag="aux")
                            v_mms(vst["pv"], kti, 0, 4)

                        def vb(kti=kti, vst=vst):
                            v_mms(vst["pv"], kti, 4, 8)
                            v_fin(vst["pv"], kti)
                        units.append(va)
                        units.append(vb)
                    else:
                        def vwhole(kti=kti):
                            pv = ps_aux.tile(
                                [128, 512], f32, name="mm", tag="aux")
                            v_mms(pv, kti, 0, 8)
                            v_fin(pv, kti)
                        units.append(vwhole)
                return units

            # ---------- out_proj units for one q-block ----------
            def outproj_units(qb, split=False):
                units = []

                def op_mms(po, ti, nb, lo, hi):
                    ats = ats_cur[qb]
                    for m in range(lo, hi):
                        nc.tensor.matmul(
                            po, ats[m][:, ti * 128:(ti + 1) * 128],
                            wo_sb[m][:, nb * 512:(nb + 1) * 512],
                            start=(m == 0), stop=(m == 3))

                def op_fin(po, ti, nb):
                    # bf16 out store: half the DMA bytes; host upcasts+sums.
                    # Mid-kernel tiles ride the sync HWDGE queue only — the
                    # gpsimd SWDGE queue carries the normalization chains and
                    # must not be delayed.
                    ob = out_p.tile([128, 512], bf16, name="ob", tag="ob")
                    nc.vector.tensor_copy(ob, po)
                    t0 = (qb * 4 + ti) * 128
                    nc.sync.dma_start(
                        out=out_d[t0:t0 + 128, nb * 512:(nb + 1) * 512], in_=ob)

                for ti in range(4):
                    for nb in range(2):
                        if split:
                            st = {}

                            def oa(ti=ti, nb=nb, st=st):
                                st["po"] = ps_aux.tile(
                                    [128, 512], f32, name="mm", tag="aux")
                                op_mms(st["po"], ti, nb, 0, 2)

                            def ob_(ti=ti, nb=nb, st=st):
                                op_mms(st["po"], ti, nb, 2, 4)
                                op_fin(st["po"], ti, nb)
                            units.append(oa)
                            units.append(ob_)
                        else:
                            def whole(ti=ti, nb=nb):
                                po = ps_aux.tile(
                                    [128, 512], f32, name="mm", tag="aux")
                                op_mms(po, ti, nb, 0, 4)
                                op_fin(po, ti, nb)
                            units.append(whole)
                return units

            # ---------- attention pair tasks + phase driver ----------
            def att_pair(qb, m, pump, fast_tail=False):
                nk = 4 * (qb + 1)
                use8 = qb >= 1   # fp8 P@V; q-block 0 stays bf16
                pvp = ps_pv.tile([65, 1024], f32, name="pv", tag="pv")

                def pv_mms(kp, pt2, stop):
                    je = 2 * kp - 4 * qb
                    pt4 = pt2.rearrange("p (s h w) -> p s h w", s=2, w=512)
                    va4 = vaug2[kp].rearrange("p (h s u) -> p h s u", s=2, u=80)
                    for h in range(2):
                        if use8 and je < 0:
                            nc.tensor.matmul(
                                pvp[:, h * 512:(h + 1) * 512],
                                va4[:, 2 * m + h, :, 0:65], pt4[:, :, h, :],
                                start=(kp == 0), stop=stop, perf_mode=DR)
                        else:
                            for s in range(2):
                                w0 = 128 * (je + s) if je + s > 0 else 0
                                lhs = (va4[:, 2 * m + h, s, 0:65] if use8 else
                                       vaugb[2 * kp + s]
                                       [:, (2 * m + h) * 65:(2 * m + h + 1) * 65])
                                nc.tensor.matmul(
                                    pvp[:, h * 512 + w0:(h + 1) * 512],
                                    lhs, pt4[:, s, h, w0:512],
                                    start=(kp == 0 and s == 0),
                                    stop=(stop and s == 1))

                def emit_S(kti):
                    j = kti - 4 * qb
                    w0 = 128 * j if j > 0 else 0
                    sp = ps_s.tile([128, 1024], f32, name="s", tag="s")
                    nc.tensor.matmul(
                        sp[:, w0:512],
                        kt[m][0:64, kti * 128:(kti + 1) * 128],
                        qt[qb][m][0:64, w0:512], start=True, stop=True)
                    nc.tensor.matmul(
                        sp[:, 512 + w0:1024],
                        kt[m][64:128, kti * 128:(kti + 1) * 128],
                        qt[qb][m][64:128, w0:512], start=True, stop=True)
                    return sp, w0, j

                prev = None
                for kp in range(nk // 2):
                    pt2 = ((p_p if use8 else p_pb)
                           .tile([128, 2048], f8 if use8 else bf16,
                                 name="p", tag="p"))
                    pt4 = pt2.rearrange("p (s h w) -> p s h w", s=2, w=512)
                    dstrip = dstrip8 if use8 else dstripb
                    for s in range(2):
                        kti = 2 * kp + s
                        sp, w0, j = emit_S(kti)
                        pump()
                        s3 = sp.rearrange("p (h w) -> p h w", w=512)
                        # q/k from fp8 DR quarters carry a 16x scale each
                        sdiv = ((16.0 if qb >= 1 else 1.0)
                                * (16.0 if kti >= 4 else 1.0))
                        nc.scalar.activation(
                            pt4[:, s, :, w0:512], s3[:, :, w0:512], Exp,
                            scale=0.125 / sdiv, bias=nbias[:, 0:1])
                        if j >= 0:
                            for h in range(2):
                                nc.vector.tensor_mul(
                                    pt4[:, s, h, w0:w0 + 128],
                                    pt4[:, s, h, w0:w0 + 128], dstrip)
                        if s == 0 and prev is not None:
                            pv_mms(*prev, stop=False)
                        pump()
                    prev = (kp, pt2)
                pv_mms(*prev, stop=True)
                if fast_tail:
                    # flush the remaining fillers BEFORE the normalization
                    # chain is emitted: the per-engine monotonic semaphores
                    # would otherwise order them behind it, idling the PE for
                    # the whole chain latency
                    for _ in range(8):
                        pump()

                # evacuate pvp fast (~2us) so ps_pv (bufs=1) recycles
                araw = at_p.tile([128, 512], f32, name=f"ar{m}", tag=f"ar{m}")
                nc.vector.tensor_copy(araw[0:64, :], pvp[0:64, 0:512])
                nc.vector.tensor_copy(araw[64:128, :], pvp[0:64, 512:1024])
                if fast_tail:
                    # latency-critical last chain: 1/Z = exp(-ln Z) on ACT
                    # (ln and exp share the natural_log_exp table set), then
                    # broadcast across partitions with two K=1 matmuls into
                    # PSUM (PE is idle here) and split the two normalization
                    # muls across DVE and Pool. ~4us vs ~13us DMA-reshape.
                    lnz = dn_p.tile([1, 1024], f32, name="lnz", tag="dn")
                    nc.scalar.activation(
                        lnz, pvp[64:65, 0:1024],
                        mybir.ActivationFunctionType.Ln)
                    rz = dn_p.tile([1, 1024], f32, name="rz", tag="rec128")
                    nc.scalar.activation(rz, lnz, Exp, scale=-1.0)
                    bcps = ps_pv.tile([128, 512], f32, name="bcps", tag="pv")
                    for h in range(2):
                        nc.tensor.matmul(
                            bcps[h * 64:(h + 1) * 64, :], ones_row,
                            rz[0:1, h * 512:(h + 1) * 512],
                            start=True, stop=True)
                    atm = at_p.tile([128, 512], bf16, name=f"at{m}",
                                    tag=f"at{m}")
                    ats_cur[qb][m] = atm
                    nc.vector.tensor_mul(atm, araw, bcps)
                    return
                bcs = bcs_p.tile([128, 512], f32, name="bcs", tag="bcs")
                if True:
                    # reciprocal at 8 elems/lane via DRAM reshape (a [1,1024]
                    # reciprocal costs ~7.9us on DVE: time scales with free
                    # size); round-trip DMAs ride the idle gpsimd SWDGE queue.
                    dmae = nc.gpsimd
                    dd2 = dram_p.tile([1, 1024], f32, name="dd2", tag="dd2")
                    dn = dn_p.tile([1, 1024], f32, name="dn", tag="dn")
                    nc.vector.tensor_copy(dn, pvp[64:65, 0:1024])
                    dd = dram_p.tile([1, 1024], f32, name="dd", tag="dd")
                    dmae.dma_start(out=dd, in_=dn)
                    den128 = dn_p.tile([128, 8], f32, name="den128",
                                       tag="den128")
                    dmae.dma_start(
                        out=den128,
                        in_=dd.rearrange("i w -> (i w)").rearrange(
                            "(p c) -> p c", c=8))
                    rec128 = dn_p.tile([128, 8], f32, name="rec128",
                                       tag="rec128")
                    nc.vector.reciprocal(rec128, den128)
                    dmae.dma_start(
                        out=dd2.rearrange("i w -> (i w)").rearrange(
                            "(p c) -> p c", c=8),
                        in_=rec128)
                    for h in range(2):
                        dmae.dma_start(
                            out=bcs[h * 64:(h + 1) * 64, :],
                            in_=dd2[0:1, h * 512:(h + 1) * 512]
                            .partition_broadcast(64))
                atm = at_p.tile([128, 512], bf16, name=f"at{m}", tag=f"at{m}")
                ats_cur[qb][m] = atm
                nc.vector.tensor_mul(atm, araw, bcs)

            def run_phase(tasks, fillers, n_units):
                """tasks: closures taking pump(); fillers pumped proportionally."""
                nf = len(fillers)
                state = {"fi": 0, "ai": 0}

                def pump():
                    state["ai"] += 1
                    while state["fi"] * n_units < state["ai"] * nf \
                            and state["fi"] < nf:
                        fillers[state["fi"]]()
                        state["fi"] += 1
                for t in tasks:
                    t(pump)
                while state["fi"] < nf:
                    fillers[state["fi"]]()
                    state["fi"] += 1

            # ---------------- emission schedule ----------------
            # warm-up matmuls on a zeroed tile fill the input-DMA window so
            # the PE clock is ramped when the first real matmul lands
            for _ in range(7):
                pw = ps_s.tile([128, 1024], f32, name="s", tag="s")
                nc.tensor.matmul(pw[:, 0:512], warm[:, 0:128], warm,
                                 start=True, stop=True)
            # pre-emit only what att(0) pair 0's scores need (qt[0][0] and
            # kt[0]); everything else in quarter 0 becomes a phase-1 filler
            # (v units first — pair 0's P@V needs them a few pumps in). This
            # puts the first exp at ~16us instead of ~43us — ACT is the
            # bottleneck engine, so its runway sets the wall.
            units0 = qkv_units(0)
            for u in (units0[0], units0[4]):
                u()
            units0_rest = [units0[i] for i in (8, 9, 10, 11, 1, 5, 2, 6, 3, 7)]
            for qb in range(NQ):
                ats_cur[qb] = [None] * 4

            def phase_tasks(qb, fast_last=False):
                def mk(m, ft):
                    def t(pump):
                        att_pair(qb, m, pump, fast_tail=ft)
                    return t
                return [mk(m, fast_last and m == 3) for m in range(4)]

            # DoubleRow makes late attention ACT-bound, so the dense PE units
            # (out_proj) are pushed as late as dependencies allow.
            # phase 1: att(0) + rest of qkv(0) + qkv(1)
            run_phase(phase_tasks(0),
                      units0_rest + qkv_units(1, split=True), 28)
            # phase 2: att(1) + qkv(2)
            run_phase(phase_tasks(1), qkv_units(2, split=True), 56)
            # phase 3: att(2) + att(3) pairs 0-2, fillers qkv(3)+op(0)+op(1)
            # spread across the whole phase (qkv(3) still lands before
            # att(3,0) starts at pump ~96 of 160)
            run_phase(phase_tasks(2) + phase_tasks(3)[:3],
                      qkv_units(3, split=True) + outproj_units(0)
                      + outproj_units(1), 140)
            # phase 4: att(3) pair 3 + op(2); pump count (32 + 8 post-PV)
            # exactly drains the fillers before the normalization chain
            run_phase(phase_tasks(3, fast_last=True)[3:], outproj_units(2), 40)
            # final out_proj, software-pipelined in two parts so the m=0..2
            # partial sums run during the last pair's normalization chain
            # (only the m=3 matmul waits on the final atm)
            ats = ats_cur[NQ - 1]
            pos = {}

            def partA(u, po):
                ti, nb = u
                for mm in range(3):
                    nc.tensor.matmul(
                        po, ats[mm][:, ti * 128:(ti + 1) * 128],
                        wo_sb[mm][:, nb * 512:(nb + 1) * 512],
                        start=(mm == 0), stop=False)
                pos[u] = po

            def partB(u):
                ti, nb = u
                po = pos.pop(u)
                nc.tensor.matmul(
                    po, ats[3][:, ti * 128:(ti + 1) * 128],
                    wo_sb[3][:, nb * 512:(nb + 1) * 512],
                    start=False, stop=True)
                ob = out_p.tile([128, 512], bf16, name="ob", tag="ob")
                nc.vector.tensor_copy(ob, po)
                t0 = ((NQ - 1) * 4 + ti) * 128
                # final tiles drain on both HWDGE queues (scalar is idle here)
                dmae = nc.sync if (ti + nb) % 2 == 0 else nc.scalar
                dmae.dma_start(
                    out=out_d[t0:t0 + 128, nb * 512:(nb + 1) * 512], in_=ob)

            units = [(ti, nb) for ti in range(4) for nb in range(2)]
            # 6 partial sums in flight: 2 on aux, 4 in the halves of the two
            # freed score tiles (scores are done) — ~18 matmuls of cover for
            # the last normalization chain's ~13us latency
            sfree1 = ps_s.tile([128, 1024], f32, name="s", tag="s")
            sfree2 = ps_s.tile([128, 1024], f32, name="s", tag="s")
            partA(units[0], ps_aux.tile([128, 512], f32, name="mm", tag="aux"))
            partA(units[1], ps_aux.tile([128, 512], f32, name="mm", tag="aux"))
            partA(units[2], sfree1[:, 0:512])
            partA(units[3], sfree1[:, 512:1024])
            partA(units[4], sfree2[:, 0:512])
            partA(units[5], sfree2[:, 512:1024])
            for i, u in enumerate(units):
                partB(u)
                if i + 6 < len(units):
                    partA(units[i + 6],
                          ps_aux.tile([128, 512], f32, name="mm", tag="aux"))
    nc.finalize()
    return nc


_NC_CACHE = {}


def _get_nc():
    if "nc" not in _NC_CACHE:
        _NC_CACHE["nc"] = build_nc()
    return _NC_CACHE["nc"]


def _make_in_maps(x, w_qkv, w_out, attn_mask):
    x = np.asarray(x, dtype=np.float32)
    w_qkv = np.asarray(w_qkv, dtype=np.float32)
    w_out = np.asarray(w_out, dtype=np.float32)
    am = np.asarray(attn_mask)
    bf = ml_dtypes.bfloat16
    f8h = ml_dtypes.float8_e4m3fn
    in_maps = []
    for c in range(NCORES):
        b, hg = c // 2, c % 2
        wqk_f = np.concatenate(
            [w_qkv[:, hg * CQ:(hg + 1) * CQ],
             w_qkv[:, DIM + hg * CQ:DIM + (hg + 1) * CQ]], axis=1)
        wqk_c = np.ascontiguousarray(wqk_f).astype(bf)
        wqk8_c = np.ascontiguousarray(16.0 * wqk_f).astype(f8h)
        wv_f = w_qkv[:, 2 * DIM + hg * CQ:2 * DIM + (hg + 1) * CQ]
        wv_c = np.ascontiguousarray(wv_f).astype(bf)
        wv8_c = np.ascontiguousarray(16.0 * wv_f).astype(f8h)
        wo_c = np.ascontiguousarray(w_out[hg * CQ:(hg + 1) * CQ, :]).astype(bf)
        mv_c = np.ascontiguousarray(
            am[b].astype(np.float32).reshape(NT, 128).T)
        xt_f = x[b].T
        xt_c = np.ascontiguousarray(xt_f[:, 0:512]).astype(bf)
        xt8_c = np.ascontiguousarray(xt_f[:, 512:]).astype(f8h)
        in_maps.append({
            "xt": xt_c,
            "xt8": xt8_c,
            "wqk": wqk_c,
            "wqk8": wqk8_c,
            "wv": wv_c,
            "wv8": wv8_c,
            "wo": wo_c,
            "maskv": mv_c,
        })
    return in_maps


def run(x, w_qkv, w_out, attn_mask, trace=False):
    nc = _get_nc()
    in_maps = _make_in_maps(x, w_qkv, w_out, attn_mask)
    res = run_bass_kernel_spmd(nc, in_maps, list(range(NCORES)), trace=trace)
    outs = [np.asarray(res.results[c]["out"]).astype(np.float32)
            for c in range(NCORES)]
    full = np.stack([outs[2 * b] + outs[2 * b + 1] for b in range(B)], axis=0)
    return full, res


def kernel(x, w_qkv, w_out, attn_mask):
    full, _ = run(x, w_qkv, w_out, attn_mask, trace=False)
    return full



# revision 45
# speedup vs baseline: 1.0196x; 1.0196x over previous
"""Causal multi-head attention block (qkv proj + attention + out proj) on 8
Trainium2 NeuronCores.

Sharding: core c = 2*b + hg handles batch b (of 4) and head-group hg (8 of 16
heads).  Each core computes qkv for its heads, causal attention, and a partial
out-projection (its 512 rows of w_out); the host sums the two head-group
partials per batch.

Design:
  - host pre-transposes x to xT [DIM, T] and pre-casts to bf16: no PE
    transposes on device, half the input DMA bytes.
  - qkv projection, scores and out_proj run in bf16 (1 cycle/row at every
    p-state and free size).  P@V runs in fp8e4m3 for q-blocks 1-3 — DoubleRow
    perf mode on off-diagonal k-tile pairs (two 128-row contraction slabs per
    matmul at 0.5 cycles/row) and plain fp8 on diagonal tiles — and in bf16
    for q-block 0, whose short attention rows would amplify fp8 quantization
    past the error budget (verified by simulation: hybrid 4.3e-3 vs 2.8e-2
    for fp8-everywhere, gate 2e-2).
  - V is stored 16x-scaled in the fp8 tiles (subnormal dodge) with a 16.0
    ones column, which cancels exactly in the softmax ratio; exp applies
    bias -3.25 so the max P (max causal score is 66.5 -> logit 8.3) stays
    under fp8e4m3's 240.
  - softmax normalization: DVE copies evacuate the PSUM accumulator fast,
    the reciprocal runs 8 elems/lane via a DRAM reshape on the idle gpsimd
    SWDGE queue, and the divide doubles as the f32->bf16 cast for out_proj.
  - emission is phase-interleaved: qkv quarter q+1 / out_proj q-1 (dense PE
    work) pump into attention block q (ACT-heavy) to keep the PE dense and
    p-state ramped; input DMA is split across both HWDGE queues with
    first-needed tiles first.
"""

import sys

if "/opt/trn_rl_repo" not in sys.path:
    sys.path.insert(0, "/opt/trn_rl_repo")

import numpy as np
import ml_dtypes

import concourse.bass as bass
import concourse.mybir as mybir
import concourse.tile as tile
from concourse import bacc
from concourse.bass_utils import run_bass_kernel_spmd

# All our activations (Exp, Ln, Copy) live in the natural_log_exp_and_others
# table set. By default the table-load pass maps Exp to exp_and_others (first
# set containing it) and Ln to natural_log_*, inserting two 1.3us table
# reloads into the latency-critical tail. Restrict the table map so every
# activation resolves to the shared set: one load at startup, none later.
_orig_get_act_tables = bacc.get_activation_tables


def _patched_get_act_tables(arch):
    tabs = _orig_get_act_tables(arch)
    keep = "natural_log_exp_and_others"
    if keep in tabs:
        ours = {mybir.ActivationFunctionType.Exp,
                mybir.ActivationFunctionType.Ln,
                mybir.ActivationFunctionType.Copy}
        tabs = {name: (fns if name == keep else fns - ours)
                for name, fns in tabs.items()}
    return tabs


bacc.get_activation_tables = _patched_get_act_tables

DIM = 1024
N_HEAD = 16
HD = 64
B, T = 4, 2048
HG = 8          # heads per core
CQ = HG * HD    # 512 feature columns per group
NCORES = 8
NT = T // 128   # 16 t-subtiles
NQ = T // 512   # 4 quarters / q-blocks

f32 = mybir.dt.float32
bf16 = mybir.dt.bfloat16
f8 = mybir.dt.float8e4
Exp = mybir.ActivationFunctionType.Exp
DR = mybir.MatmulPerfMode.DoubleRow
EXP_BIAS = -3.25


def build_nc():
    nc = bacc.Bacc(None, target_bir_lowering=False)
    # bf16 x columns t 0..511 only (quarter 0); rows t>=512 project in fp8
    xt_d = nc.declare_dram_parameter("xt", [DIM, 512], bf16, isOutput=False)
    xt8_d = nc.declare_dram_parameter("xt8", [DIM, T - 512], f8, isOutput=False)
    wqk_d = nc.declare_dram_parameter("wqk", [DIM, 2 * CQ], bf16, isOutput=False)
    wqk8_d = nc.declare_dram_parameter("wqk8", [DIM, 2 * CQ], f8,
                                       isOutput=False)
    wv_d = nc.declare_dram_parameter("wv", [DIM, CQ], bf16, isOutput=False)
    wv8_d = nc.declare_dram_parameter("wv8", [DIM, CQ], f8, isOutput=False)
    wo_d = nc.declare_dram_parameter("wo", [CQ, DIM], bf16, isOutput=False)
    mv_d = nc.declare_dram_parameter("maskv", [128, NT], f32, isOutput=False)
    out_d = nc.declare_dram_parameter("out", [T, DIM], bf16, isOutput=True)

    with tile.TileContext(nc) as tc:
        with tc.tile_pool(name="pp", bufs=1) as pp, \
             tc.tile_pool(name="p_p", bufs=6) as p_p, \
             tc.tile_pool(name="p_pb", bufs=6) as p_pb, \
             tc.tile_pool(name="at_p", bufs=2) as at_p, \
             tc.tile_pool(name="dn_p", bufs=3) as dn_p, \
             tc.tile_pool(name="bcs_p", bufs=2) as bcs_p, \
             tc.tile_pool(name="out_p", bufs=4) as out_p, \
             tc.tile_pool(name="dram_p", bufs=3, space="DRAM") as dram_p, \
             tc.tile_pool(name="ps_aux", bufs=2, space="PSUM") as ps_aux, \
             tc.tile_pool(name="ps_s", bufs=2, space="PSUM") as ps_s, \
             tc.tile_pool(name="ps_pv", bufs=1, space="PSUM") as ps_pv:

            # ---- persistent input tiles ----
            # merged big tiles + per-k views: each input lands in ONE or TWO
            # big multi-tile DMAs. The scalar HWDGE queue gets only 5 trigger
            # instructions, so the first ACTIVATE isn't stuck behind a
            # 40-trigger FIFO (measured: exp start 46us -> ~16us).
            TQ = T - 512
            xbig = pp.tile([128, 8 * 512], bf16, name="xbig", tag="xbig")
            xts = [xbig[:, kb * 512:(kb + 1) * 512] for kb in range(8)]
            wqkbig = pp.tile([128, 8 * 2 * CQ], bf16, name="wqkbig",
                             tag="wqkbig")
            wqk_sb = [wqkbig[:, k * 2 * CQ:(k + 1) * 2 * CQ] for k in range(8)]
            wvbig = pp.tile([128, 8 * CQ], bf16, name="wvbig", tag="wvbig")
            wv_sb = [wvbig[:, k * CQ:(k + 1) * CQ] for k in range(8)]
            wobig = pp.tile([128, 4 * DIM], bf16, name="wobig", tag="wobig")
            wo_sb = [wobig[:, m * DIM:(m + 1) * DIM] for m in range(4)]
            mv_sb = pp.tile([128, NT], f32, name="maskv_sb", tag="maskv_sb")
            # fp8 DoubleRow pair tiles (dim-block pairs on the slab axis):
            # 16x-scaled weights; x rows t>=512
            x8big = pp.tile([128, 8 * TQ], f8, name="x8big", tag="x8big")
            x8v = [x8big[:, kp * 2 * TQ:(kp + 1) * 2 * TQ]
                   .rearrange("p (s c) -> p s c", c=TQ) for kp in range(4)]
            wqk8big = pp.tile([128, 8 * 2 * CQ], f8, name="wqk8big",
                              tag="wqk8big")
            wqk8v = [wqk8big[:, kp * 4 * CQ:(kp + 1) * 4 * CQ]
                     .rearrange("p (s c) -> p s c", c=2 * CQ)
                     for kp in range(4)]
            wv8big = pp.tile([128, 8 * CQ], f8, name="wv8big", tag="wv8big")
            wv8v = [wv8big[:, kp * 2 * CQ:(kp + 1) * 2 * CQ]
                    .rearrange("p (s c) -> p s c", c=CQ) for kp in range(4)]

            # big-tile (k, p, c) views for the batched DMAs
            xbig3 = xbig.rearrange("p (k c) -> p k c", c=512)
            xd3 = xt_d.rearrange("(k p) c -> p k c", p=128)
            wqkbig3 = wqkbig.rearrange("p (k c) -> p k c", c=2 * CQ)
            wqkd3 = wqk_d.rearrange("(k p) c -> p k c", p=128)
            wvbig3 = wvbig.rearrange("p (k c) -> p k c", c=CQ)
            wvd3 = wv_d.rearrange("(k p) c -> p k c", p=128)
            x8big3 = x8big.rearrange("p (j c) -> p j c", c=TQ)
            x8d3 = xt8_d.rearrange("(j p) c -> p j c", p=128)
            wqk8big3 = wqk8big.rearrange("p (j c) -> p j c", c=2 * CQ)
            wqk8d3 = wqk8_d.rearrange("(j p) c -> p j c", p=128)
            wv8big3 = wv8big.rearrange("p (j c) -> p j c", c=CQ)
            wv8d3 = wv8_d.rearrange("(j p) c -> p j c", p=128)

            # warm tile: the warm-up matmuls only need to occupy the PE
            # (outputs are discarded; emit_S start=True clears the PSUM banks
            # before real use). A 1-element DMA write satisfies the tile
            # allocator without a full memset on the critical path; it must
            # be the FIRST sync transfer so the warm MMs start immediately.
            warm = pp.tile([128, 512], bf16, name="warm", tag="warm")
            nc.sync.dma_start(out=warm[0:1, 0:1], in_=xt_d[0:1, 0:1])
            # scalar queue: 5 early triggers only (maskv + its half of the
            # startup burst); everything else rides sync in first-needed order
            nc.scalar.dma_start(out=mv_sb, in_=mv_d[:, :])
            nc.sync.dma_start(out=wqkbig3[:, 0:4, 0:128],
                              in_=wqkd3[:, 0:4, 0:128])
            nc.scalar.dma_start(out=wqkbig3[:, 4:8, 0:128],
                                in_=wqkd3[:, 4:8, 0:128])
            nc.sync.dma_start(out=xbig3[:, 0:4, :], in_=xd3[:, 0:4, :])
            nc.scalar.dma_start(out=xbig3[:, 4:8, :], in_=xd3[:, 4:8, :])
            # k-projection m=4 columns next: the first score matmul needs
            # kt[0], which is unit m4's output
            nc.sync.dma_start(out=wqkbig3[:, 0:4, 512:640],
                              in_=wqkd3[:, 0:4, 512:640])
            nc.scalar.dma_start(out=wqkbig3[:, 4:8, 512:640],
                                in_=wqkd3[:, 4:8, 512:640])
            # wv before the wqk remainders: the v units are the first
            # phase-1 fillers
            nc.sync.dma_start(out=wvbig3[:, 0:4, :], in_=wvd3[:, 0:4, :])
            nc.scalar.dma_start(out=wvbig3[:, 4:8, :], in_=wvd3[:, 4:8, :])
            # wqk remainders all on sync: scalar's FIFO must clear so the
            # act-table load + first ACTIVATEs aren't queued behind triggers
            nc.sync.dma_start(out=wqkbig3[:, :, 128:512],
                              in_=wqkd3[:, :, 128:512])
            nc.sync.dma_start(out=wqkbig3[:, :, 640:2 * CQ],
                              in_=wqkd3[:, :, 640:2 * CQ])
            nc.sync.dma_start(out=wqk8big3[:, :, 0:128],
                              in_=wqk8d3[:, :, 0:128])
            nc.sync.dma_start(out=x8big3[:, :, 0:512], in_=x8d3[:, :, 0:512])
            nc.sync.dma_start(out=wqk8big3[:, :, 128:2 * CQ],
                              in_=wqk8d3[:, :, 128:2 * CQ])
            nc.sync.dma_start(out=wv8big3[:, :, :], in_=wv8d3[:, :, :])
            nc.sync.dma_start(out=x8big3[:, :, 512:TQ], in_=x8d3[:, :, 512:TQ])
            nc.sync.dma_start(out=wobig.rearrange("p (k c) -> p k c", c=DIM),
                              in_=wo_d.rearrange("(k p) c -> p k c", p=128))
            # dummy activation: hoists the 1.3us ACT table load into the
            # startup window instead of serializing before the first real exp
            actwarm = pp.tile([1, 1], f32, name="actwarm", tag="actwarm")
            nc.scalar.activation(actwarm, mv_sb[0:1, 0:1],
                                 mybir.ActivationFunctionType.Copy)

            # ---- constants ----
            # one 128x128 causal strip: keep where q_local >= k_local
            dstrip32 = pp.tile([128, 128], f32, name="dstrip32", tag="dstrip32")
            nc.gpsimd.memset(dstrip32, 1.0)
            nc.gpsimd.affine_select(
                out=dstrip32, in_=dstrip32, compare_op=mybir.AluOpType.is_ge,
                fill=0.0, base=0, pattern=[[1, 128]], channel_multiplier=-1)
            dstrip8 = pp.tile([128, 128], f8, name="dstrip8", tag="dstrip8")
            nc.vector.tensor_copy(dstrip8, dstrip32)
            dstripb = pp.tile([128, 128], bf16, name="dstripb", tag="dstripb")
            nc.vector.tensor_copy(dstripb, dstrip32)
            # fp8 V path: values 16x-scaled, ones column 16.0
            ones16 = pp.tile([128, HG], f32, name="ones16", tag="ones16")
            nc.vector.memset(ones16, 16.0)
            ones1 = pp.tile([128, HG], f32, name="ones1", tag="ones1")
            nc.vector.memset(ones1, 1.0)
            mv16_sb = pp.tile([128, NT], f32, name="mv16", tag="mv16")
            nc.vector.tensor_scalar_mul(mv16_sb, mv_sb, 16.0)
            # exp bias (fp8e4m3 range headroom); cancels in the softmax
            nbias = pp.tile([128, 1], f32, name="nbias", tag="nbias")
            nc.vector.memset(nbias, EXP_BIAS)
            # [1, 64] ones row: K=1 matmul broadcasts 1/Z across partitions
            ones_row = pp.tile([1, 64], f32, name="ones_row", tag="ones_row")
            nc.vector.memset(ones_row, 1.0)

            # ---- persistent compute tensors ----
            kt = [pp.tile([128, T], bf16, name=f"kt{m}", tag=f"kt{m}")
                  for m in range(4)]
            # fp8 V tiles for DoubleRow: vaug2[kp] holds k-tiles (2kp, 2kp+1)
            # laid out (head, slab, 64 V cols + ones col + 15 pad): the slab
            # stride must be a 16B multiple for dual-fp8 ldweights
            vaug2 = [pp.tile([128, 2 * HG * 80], f8, name=f"va{t}", tag=f"va{t}")
                     for t in range(NT // 2)]
            # bf16 V tiles for q-block 0 (k-tiles 0-3 only), (head, 65) layout
            vaugb = [pp.tile([128, HG * 65], bf16, name=f"vb{t}", tag=f"vb{t}")
                     for t in range(4)]
            qt = [[pp.tile([128, 512], bf16, name=f"qt{q}_{m}", tag=f"qt{q}_{m}")
                   for m in range(4)] for q in range(NQ)]

            ats_cur = {}   # qb -> [4 pair tiles [128, 512] bf16]

            # ---------- qkv quarter units (each ~1.7us of PE) ----------
            # split=True emits each qk unit as two 4-matmul halves (the PSUM
            # accumulation group spans the split): finer filler grain in the
            # ACT-bound phase 3 so attention score matmuls queue sooner
            def qkv_units(q, split=False):
                units = []
                dr = q >= 1   # fp8 DoubleRow projection for rows t >= 512

                def qk_mms(pq, m, lo, hi):
                    if dr:
                        # halves map to slab pairs: (lo,hi) in MM units of 1
                        for kp in range(lo // 2, hi // 2):
                            nc.tensor.matmul(
                                pq, wqk8v[kp][:, :, m * 128:(m + 1) * 128],
                                x8v[kp][:, :, (q - 1) * 512:q * 512],
                                start=(kp == 0), stop=(kp == 3), perf_mode=DR)
                        return
                    # kb order matches DMA arrival (sync/scalar interleaved)
                    # so quarter 0 isn't gated on the whole startup burst
                    order = (0, 4, 1, 5, 2, 6, 3, 7)
                    for i in range(lo, hi):
                        nc.tensor.matmul(
                            pq, wqk_sb[order[i]][:, m * 128:(m + 1) * 128],
                            xts[order[i]][:, 0:512],
                            start=(i == 0), stop=(i == 7))

                def qk_fin(pq, m):
                    if m < 4:
                        nc.vector.tensor_copy(qt[q][m], pq)
                    else:
                        nc.vector.tensor_copy(
                            kt[m - 4][:, q * 512:(q + 1) * 512], pq)

                for m in range(8):
                    if split:
                        st = {}

                        def ha(m=m, st=st):
                            st["pq"] = ps_aux.tile(
                                [128, 512], f32, name="mm", tag="aux")
                            qk_mms(st["pq"], m, 0, 4)

                        def hb(m=m, st=st):
                            qk_mms(st["pq"], m, 4, 8)
                            qk_fin(st["pq"], m)
                        units.append(ha)
                        units.append(hb)
                    else:
                        def whole(m=m):
                            pq = ps_aux.tile(
                                [128, 512], f32, name="mm", tag="aux")
                            qk_mms(pq, m, 0, 8)
                            qk_fin(pq, m)
                        units.append(whole)

                def v_mms(pv, kti, lo, hi):
                    if dr:
                        for kp in range(lo // 2, hi // 2):
                            nc.tensor.matmul(
                                pv,
                                x8v[kp][:, :, (kti - 4) * 128:(kti - 3) * 128],
                                wv8v[kp], start=(kp == 0), stop=(kp == 3),
                                perf_mode=DR)
                        return
                    for kb in range(lo, hi):
                        nc.tensor.matmul(
                            pv, xts[kb][:, kti * 128:(kti + 1) * 128], wv_sb[kb],
                            start=(kb == 0), stop=(kb == 7))

                def v_fin(pv, kti):
                    pv3 = pv.rearrange("p (h w) -> p h w", w=64)
                    vt4 = vaug2[kti // 2].rearrange(
                        "p (h s u) -> p h s u", s=2, u=80)
                    s = kti % 2
                    # DR quarters produce 16x-scaled v already: mask only
                    vscale = mv_sb if dr else mv16_sb
                    nc.vector.tensor_scalar_mul(
                        vt4[:, :, s, 0:64], pv3, vscale[:, kti:kti + 1])
                    nc.vector.tensor_scalar_mul(
                        vt4[:, :, s, 64:65],
                        ones16.rearrange("p (h w) -> p h w", w=1),
                        mv_sb[:, kti:kti + 1])
                    if q == 0:
                        vb3 = vaugb[kti].rearrange("p (h u) -> p h u", u=65)
                        nc.vector.tensor_scalar_mul(
                            vb3[:, :, 0:64], pv3, mv_sb[:, kti:kti + 1])
                        nc.vector.tensor_scalar_mul(
                            vb3[:, :, 64:65],
                            ones1.rearrange("p (h w) -> p h w", w=1),
                            mv_sb[:, kti:kti + 1])

                for ti in range(4):
                    kti = q * 4 + ti
                    if split:
                        vst = {}

                        def va(kti=kti, vst=vst):
                            vst["pv"] = ps_aux.tile(
                                [128, 512], f32, name="mm", tag="aux")
                            v_mms(vst["pv"], kti, 0, 4)

                        def vb(kti=kti, vst=vst):
                            v_mms(vst["pv"], kti, 4, 8)
                            v_fin(vst["pv"], kti)
                        units.append(va)
                        units.append(vb)
                    else:
                        def vwhole(kti=kti):
                            pv = ps_aux.tile(
                                [128, 512], f32, name="mm", tag="aux")
                            v_mms(pv, kti, 0, 8)
                            v_fin(pv, kti)
                        units.append(vwhole)
                return units

            # ---------- out_proj units for one q-block ----------
            def outproj_units(qb, split=False):
                units = []

                def op_mms(po, ti, nb, lo, hi):
                    ats = ats_cur[qb]
                    for m in range(lo, hi):
                        nc.tensor.matmul(
                            po, ats[m][:, ti * 128:(ti + 1) * 128],
                            wo_sb[m][:, nb * 512:(nb + 1) * 512],
                            start=(m == 0), stop=(m == 3))

                def op_fin(po, ti, nb):
                    # bf16 out store: half the DMA bytes; host upcasts+sums.
                    # Mid-kernel tiles ride the sync HWDGE queue only — the
                    # gpsimd SWDGE queue carries the normalization chains and
                    # must not be delayed.
                    ob = out_p.tile([128, 512], bf16, name="ob", tag="ob")
                    nc.vector.tensor_copy(ob, po)
                    t0 = (qb * 4 + ti) * 128
                    nc.sync.dma_start(
                        out=out_d[t0:t0 + 128, nb * 512:(nb + 1) * 512], in_=ob)

                for ti in range(4):
                    for nb in range(2):
                        if split:
                            st = {}

                            def oa(ti=ti, nb=nb, st=st):
                                st["po"] = ps_aux.tile(
                                    [128, 512], f32, name="mm", tag="aux")
                                op_mms(st["po"], ti, nb, 0, 2)

                            def ob_(ti=ti, nb=nb, st=st):
                                op_mms(st["po"], ti, nb, 2, 4)
                                op_fin(st["po"], ti, nb)
                            units.append(oa)
                            units.append(ob_)
                        else:
                            def whole(ti=ti, nb=nb):
                                po = ps_aux.tile(
                                    [128, 512], f32, name="mm", tag="aux")
                                op_mms(po, ti, nb, 0, 4)
                                op_fin(po, ti, nb)
                            units.append(whole)
                return units

            # ---------- attention pair tasks + phase driver ----------
            def att_pair(qb, m, pump, fast_tail=False):
                nk = 4 * (qb + 1)
                use8 = qb >= 1   # fp8 P@V; q-block 0 stays bf16
                pvp = ps_pv.tile([65, 1024], f32, name="pv", tag="pv")

                def pv_mms(kp, pt2, stop):
                    je = 2 * kp - 4 * qb
                    pt4 = pt2.rearrange("p (s h w) -> p s h w", s=2, w=512)
                    va4 = vaug2[kp].rearrange("p (h s u) -> p h s u", s=2, u=80)
                    for h in range(2):
                        if use8 and je < 0:
                            nc.tensor.matmul(
                                pvp[:, h * 512:(h + 1) * 512],
                                va4[:, 2 * m + h, :, 0:65], pt4[:, :, h, :],
                                start=(kp == 0), stop=stop, perf_mode=DR)
                        else:
                            for s in range(2):
                                w0 = 128 * (je + s) if je + s > 0 else 0
                                lhs = (va4[:, 2 * m + h, s, 0:65] if use8 else
                                       vaugb[2 * kp + s]
                                       [:, (2 * m + h) * 65:(2 * m + h + 1) * 65])
                                nc.tensor.matmul(
                                    pvp[:, h * 512 + w0:(h + 1) * 512],
                                    lhs, pt4[:, s, h, w0:512],
                                    start=(kp == 0 and s == 0),
                                    stop=(stop and s == 1))

                def emit_S(kti):
                    j = kti - 4 * qb
                    w0 = 128 * j if j > 0 else 0
                    sp = ps_s.tile([128, 1024], f32, name="s", tag="s")
                    nc.tensor.matmul(
                        sp[:, w0:512],
                        kt[m][0:64, kti * 128:(kti + 1) * 128],
                        qt[qb][m][0:64, w0:512], start=True, stop=True)
                    nc.tensor.matmul(
                        sp[:, 512 + w0:1024],
                        kt[m][64:128, kti * 128:(kti + 1) * 128],
                        qt[qb][m][64:128, w0:512], start=True, stop=True)
                    return sp, w0, j

                prev = None
                for kp in range(nk // 2):
                    pt2 = ((p_p if use8 else p_pb)
                           .tile([128, 2048], f8 if use8 else bf16,
                                 name="p", tag="p"))
                    pt4 = pt2.rearrange("p (s h w) -> p s h w", s=2, w=512)
                    dstrip = dstrip8 if use8 else dstripb
                    for s in range(2):
                        kti = 2 * kp + s
                        sp, w0, j = emit_S(kti)
                        pump()
                        s3 = sp.rearrange("p (h w) -> p h w", w=512)
                        # q/k from fp8 DR quarters carry a 16x scale each
                        sdiv = ((16.0 if qb >= 1 else 1.0)
                                * (16.0 if kti >= 4 else 1.0))
                        nc.scalar.activation(
                            pt4[:, s, :, w0:512], s3[:, :, w0:512], Exp,
                            scale=0.125 / sdiv, bias=nbias[:, 0:1])
                        if j >= 0:
                            for h in range(2):
                                nc.vector.tensor_mul(
                                    pt4[:, s, h, w0:w0 + 128],
                                    pt4[:, s, h, w0:w0 + 128], dstrip)
                        if s == 0 and prev is not None:
                            pv_mms(*prev, stop=False)
                        pump()
                    prev = (kp, pt2)
                pv_mms(*prev, stop=True)
                if fast_tail:
                    # flush the remaining fillers BEFORE the normalization
                    # chain is emitted: the per-engine monotonic semaphores
                    # would otherwise order them behind it, idling the PE for
                    # the whole chain latency
                    for _ in range(8):
                        pump()

                # evacuate pvp fast (~2us) so ps_pv (bufs=1) recycles
                araw = at_p.tile([128, 512], f32, name=f"ar{m}", tag=f"ar{m}")
                nc.vector.tensor_copy(araw[0:64, :], pvp[0:64, 0:512])
                nc.vector.tensor_copy(araw[64:128, :], pvp[0:64, 512:1024])
                if fast_tail:
                    # latency-critical last chain: 1/Z = exp(-ln Z) on ACT
                    # (ln and exp share the natural_log_exp table set), then
                    # broadcast across partitions with two K=1 matmuls into
                    # PSUM (PE is idle here) and split the two normalization
                    # muls across DVE and Pool. ~4us vs ~13us DMA-reshape.
                    lnz = dn_p.tile([1, 1024], f32, name="lnz", tag="dn")
                    nc.scalar.activation(
                        lnz, pvp[64:65, 0:1024],
                        mybir.ActivationFunctionType.Ln)
                    rz = dn_p.tile([1, 1024], f32, name="rz", tag="rec128")
                    nc.scalar.activation(rz, lnz, Exp, scale=-1.0)
                    bcps = ps_pv.tile([128, 512], f32, name="bcps", tag="pv")
                    for h in range(2):
                        nc.tensor.matmul(
                            bcps[h * 64:(h + 1) * 64, :], ones_row,
                            rz[0:1, h * 512:(h + 1) * 512],
                            start=True, stop=True)
                    atm = at_p.tile([128, 512], bf16, name=f"at{m}",
                                    tag=f"at{m}")
                    ats_cur[qb][m] = atm
                    nc.vector.tensor_mul(atm, araw, bcps)
                    return
                bcs = bcs_p.tile([128, 512], f32, name="bcs", tag="bcs")
                if True:
                    # reciprocal at 8 elems/lane via DRAM reshape (a [1,1024]
                    # reciprocal costs ~7.9us on DVE: time scales with free
                    # size); round-trip DMAs ride the idle gpsimd SWDGE queue.
                    dmae = nc.gpsimd
                    dd2 = dram_p.tile([1, 1024], f32, name="dd2", tag="dd2")
                    dn = dn_p.tile([1, 1024], f32, name="dn", tag="dn")
                    nc.vector.tensor_copy(dn, pvp[64:65, 0:1024])
                    dd = dram_p.tile([1, 1024], f32, name="dd", tag="dd")
                    dmae.dma_start(out=dd, in_=dn)
                    den128 = dn_p.tile([128, 8], f32, name="den128",
                                       tag="den128")
                    dmae.dma_start(
                        out=den128,
                        in_=dd.rearrange("i w -> (i w)").rearrange(
                            "(p c) -> p c", c=8))
                    rec128 = dn_p.tile([128, 8], f32, name="rec128",
                                       tag="rec128")
                    nc.vector.reciprocal(rec128, den128)
                    dmae.dma_start(
                        out=dd2.rearrange("i w -> (i w)").rearrange(
                            "(p c) -> p c", c=8),
                        in_=rec128)
                    for h in range(2):
                        dmae.dma_start(
                            out=bcs[h * 64:(h + 1) * 64, :],
                            in_=dd2[0:1, h * 512:(h + 1) * 512]
                            .partition_broadcast(64))
                atm = at_p.tile([128, 512], bf16, name=f"at{m}", tag=f"at{m}")
                ats_cur[qb][m] = atm
                nc.vector.tensor_mul(atm, araw, bcs)

            def run_phase(tasks, fillers, n_units):
                """tasks: closures taking pump(); fillers pumped proportionally."""
                nf = len(fillers)
                state = {"fi": 0, "ai": 0}

                def pump():
                    state["ai"] += 1
                    while state["fi"] * n_units < state["ai"] * nf \
                            and state["fi"] < nf:
                        fillers[state["fi"]]()
                        state["fi"] += 1
                for t in tasks:
                    t(pump)
                while state["fi"] < nf:
                    fillers[state["fi"]]()
                    state["fi"] += 1

            # ---------------- emission schedule ----------------
            # warm-up matmuls on a zeroed tile fill the input-DMA window so
            # the PE clock is ramped when the first real matmul lands
            for _ in range(7):
                pw = ps_s.tile([128, 1024], f32, name="s", tag="s")
                nc.tensor.matmul(pw[:, 0:512], warm[:, 0:128], warm,
                                 start=True, stop=True)
            # pre-emit only what att(0) pair 0's scores need (qt[0][0] and
            # kt[0]); everything else in quarter 0 becomes a phase-1 filler
            # (v units first — pair 0's P@V needs them a few pumps in). This
            # puts the first exp at ~16us instead of ~43us — ACT is the
            # bottleneck engine, so its runway sets the wall.
            units0 = qkv_units(0)
            for u in (units0[0], units0[4]):
                u()
            units0_rest = [units0[i] for i in (8, 9, 10, 11, 1, 5, 2, 6, 3, 7)]
            for qb in range(NQ):
                ats_cur[qb] = [None] * 4

            def phase_tasks(qb, fast_last=False):
                def mk(m, ft):
                    def t(pump):
                        att_pair(qb, m, pump, fast_tail=ft)
                    return t
                return [mk(m, fast_last and m == 3) for m in range(4)]

            # DoubleRow makes late attention ACT-bound, so the dense PE units
            # (out_proj) are pushed as late as dependencies allow.
            # phase 1: att(0) + rest of qkv(0) + qkv(1)
            run_phase(phase_tasks(0),
                      units0_rest + qkv_units(1, split=True), 28)
            # phase 2: att(1) + qkv(2)
            run_phase(phase_tasks(1), qkv_units(2, split=True), 56)
            # phase 3: att(2) + att(3) pairs 0-2, fillers qkv(3)+op(0)+op(1)
            # spread across the whole phase (qkv(3) still lands before
            # att(3,0) starts at pump ~96 of 160)
            run_phase(phase_tasks(2) + phase_tasks(3)[:3],
                      qkv_units(3, split=True) + outproj_units(0)
                      + outproj_units(1), 140)
            # phase 4: att(3) pair 3 + op(2); pump count (32 + 8 post-PV)
            # exactly drains the fillers before the normalization chain
            run_phase(phase_tasks(3, fast_last=True)[3:], outproj_units(2), 40)
            # final out_proj, software-pipelined in two parts so the m=0..2
            # partial sums run during the last pair's normalization chain
            # (only the m=3 matmul waits on the final atm)
            ats = ats_cur[NQ - 1]
            pos = {}

            def partA(u, po):
                ti, nb = u
                for mm in range(3):
                    nc.tensor.matmul(
                        po, ats[mm][:, ti * 128:(ti + 1) * 128],
                        wo_sb[mm][:, nb * 512:(nb + 1) * 512],
                        start=(mm == 0), stop=False)
                pos[u] = po

            def partB(u):
                ti, nb = u
                po = pos.pop(u)
                nc.tensor.matmul(
                    po, ats[3][:, ti * 128:(ti + 1) * 128],
                    wo_sb[3][:, nb * 512:(nb + 1) * 512],
                    start=False, stop=True)
                ob = out_p.tile([128, 512], bf16, name="ob", tag="ob")
                nc.vector.tensor_copy(ob, po)
                t0 = ((NQ - 1) * 4 + ti) * 128
                # final tiles drain on both HWDGE queues (scalar is idle here)
                dmae = nc.sync if (ti + nb) % 2 == 0 else nc.scalar
                dmae.dma_start(
                    out=out_d[t0:t0 + 128, nb * 512:(nb + 1) * 512], in_=ob)

            units = [(ti, nb) for ti in range(4) for nb in range(2)]
            # 6 partial sums in flight: 2 on aux, 4 in the halves of the two
            # freed score tiles (scores are done) — ~18 matmuls of cover for
            # the last normalization chain's ~13us latency
            sfree1 = ps_s.tile([128, 1024], f32, name="s", tag="s")
            sfree2 = ps_s.tile([128, 1024], f32, name="s", tag="s")
            partA(units[0], ps_aux.tile([128, 512], f32, name="mm", tag="aux"))
            partA(units[1], ps_aux.tile([128, 512], f32, name="mm", tag="aux"))
            partA(units[2], sfree1[:, 0:512])
            partA(units[3], sfree1[:, 512:1024])
            partA(units[4], sfree2[:, 0:512])
            partA(units[5], sfree2[:, 512:1024])
            for i, u in enumerate(units):
                partB(u)
                if i + 6 < len(units):
                    partA(units[i + 6],
                          ps_aux.tile([128, 512], f32, name="mm", tag="aux"))
    nc.finalize()
    return nc


_NC_CACHE = {}


def _get_nc():
    if "nc" not in _NC_CACHE:
        _NC_CACHE["nc"] = build_nc()
    return _NC_CACHE["nc"]


def _make_in_maps(x, w_qkv, w_out, attn_mask):
    x = np.asarray(x, dtype=np.float32)
    w_qkv = np.asarray(w_qkv, dtype=np.float32)
    w_out = np.asarray(w_out, dtype=np.float32)
    am = np.asarray(attn_mask)
    bf = ml_dtypes.bfloat16
    f8h = ml_dtypes.float8_e4m3fn
    in_maps = []
    for c in range(NCORES):
        b, hg = c // 2, c % 2
        wqk_f = np.concatenate(
            [w_qkv[:, hg * CQ:(hg + 1) * CQ],
             w_qkv[:, DIM + hg * CQ:DIM + (hg + 1) * CQ]], axis=1)
        wqk_c = np.ascontiguousarray(wqk_f).astype(bf)
        wqk8_c = np.ascontiguousarray(16.0 * wqk_f).astype(f8h)
        wv_f = w_qkv[:, 2 * DIM + hg * CQ:2 * DIM + (hg + 1) * CQ]
        wv_c = np.ascontiguousarray(wv_f).astype(bf)
        wv8_c = np.ascontiguousarray(16.0 * wv_f).astype(f8h)
        wo_c = np.ascontiguousarray(w_out[hg * CQ:(hg + 1) * CQ, :]).astype(bf)
        mv_c = np.ascontiguousarray(
            am[b].astype(np.float32).reshape(NT, 128).T)
        xt_f = x[b].T
        xt_c = np.ascontiguousarray(xt_f[:, 0:512]).astype(bf)
        xt8_c = np.ascontiguousarray(xt_f[:, 512:]).astype(f8h)
        in_maps.append({
            "xt": xt_c,
            "xt8": xt8_c,
            "wqk": wqk_c,
            "wqk8": wqk8_c,
            "wv": wv_c,
            "wv8": wv8_c,
            "wo": wo_c,
            "maskv": mv_c,
        })
    return in_maps


def run(x, w_qkv, w_out, attn_mask, trace=False):
    nc = _get_nc()
    in_maps = _make_in_maps(x, w_qkv, w_out, attn_mask)
    res = run_bass_kernel_spmd(nc, in_maps, list(range(NCORES)), trace=trace)
    outs = [np.asarray(res.results[c]["out"]).astype(np.float32)
            for c in range(NCORES)]
    full = np.stack([outs[2 * b] + outs[2 * b + 1] for b in range(B)], axis=0)
    return full, res


def kernel(x, w_qkv, w_out, attn_mask):
    full, _ = run(x, w_qkv, w_out, attn_mask, trace=False)
    return full

